# revision 23
# baseline (speedup 1.0000x reference)
"""Trainium2 Bass kernel for nn_ContrastiveSWM (GNN message passing).

Math (per reference.py):
  B=1024, N=8, D=256, H=512, A=4.  flat = states.reshape(B*N, D)
  Edge list over N-1=7 "virtual" objects with stride 7: flat rows
  [7g, 7g+7) form 1024 independent fully-connected 7-node blocks
  (rows 0..7167); rows 7168..8191 have no edges.
  8 rounds of: e = edgeMLP([flat[row], node[row], flat[col]]);
  agg = segsum(e, row); node = nodeMLP([flat, av, agg]); flat += node.

Key algebraic restructurings (exact, not approximations):
  * edge MLP layer 1 splits: u_i = flat_i@w1a + node_i@w1b (+b1),
    v_j = flat_j@w1c; per-edge preact = u_i + v_j.
  * edge MLP layer 3 + segment-sum commute: agg_i = (sum_j h2_ij)@e_w3
    (+ 6*e_b3), and e_w3 further folds into the node MLP:
    agg@n_w1g = s@(e_w3@n_w1g).  So only layer 2 is per-edge.
  * LN mean folds into the layer-2 weights (biases are zero in the fast
    path): z2 - mean = h1 @ (e_w2 - rowsum(e_w2)/H), exactly.  var =
    mean of (z2c)^2 via ones/H matmul, rstd in one ACT op
    (Abs_reciprocal_sqrt; single act-table set).

Sharding: data-parallel over the 1024 edge-blocks: core d owns blocks
[128d,128(d+1)) = flat rows [896d, 896(d+1)), plus tail rows
[7168+128d, 7168+128(d+1)).  Weights replicated.  No collectives.

On-device layout is feature-major (features on partitions, rows in the
free dim), slot-major columns (col = slot*128 + block) so that the
fully-connected gather becomes contiguous 128-column slices.

Dtypes: residual stream (flat/node) and layer-1 weights in float32r
(full-rate PE at free-dim>=256, ~tf32 accuracy); all other activations
and weights in float16 (fast DVE modes, 8x finer mantissa than bf16);
PSUM accumulation fp32.  Measured absmax-relative error vs the fp32
reference: 7.2e-4.  The edge loop is software-pipelined (stage B of
slot i emitted after stage A of slot i+1) because each engine executes
its instruction stream in order.
"""

import numpy as np

try:
    import concourse.bass as bass
except ImportError:  # environment fallback
    import sys

    sys.path.insert(0, "/opt/trn_rl_repo")
    import concourse.bass as bass

import concourse.mybir as mybir
import concourse.tile as tile
from concourse import bacc, bass_utils

F32 = mybir.dt.float32
F32R = mybir.dt.float32r
BF16 = mybir.dt.float16  # fp16: same speed class as bf16, 8x finer mantissa
F8 = mybir.dt.float8e4  # TRN e4m3 (max 240); enables DoubleRow 2x matmul
DR = mybir.MatmulPerfMode.DoubleRow
AL = mybir.AluOpType
AF = mybir.ActivationFunctionType

# fp8 scaling scheme (all exact, compensated):
#  * Square ops use scale=1/16 so sq values fit fp8e4 range.
#  * wsg, nw2 are scaled x16 on host (keeps fp8 weights out of denormals);
#    the LN rstd ACT op absorbs the compensation:
#      edge: rstd_stored = rstd/16  (so s=sum h2/16 pairs with 16*wsg)
#      node: t_n = 16*z2c, rstd_stored = rstd/16 (relu commutes with scale)
SQS = 0.0625  # square input prescale

B, N, D, H, A = 1024, 8, 256, 512, 4
M = 8  # cores
NB = 128  # blocks per core
S = N - 1  # 7 slots per block
EC = NB * S  # 896 edge cols per core
TC = (B * N - B * S) // M  # 128 tail cols per core
C = EC + TC  # 1024 cols per core
ROUNDS = N  # 8
EPS = 1e-5
ND = D // 128  # 2
NH = H // 128  # 4

_CACHE = {}


def _bc(ap2d, n, w):
    """[p, w] -> [p, n, w] broadcast along a new middle dim."""
    return ap2d.unsqueeze(1).to_broadcast((ap2d.shape[0], n, w))


def _force_single_act_set(nc):
    """All activation funcs we use live in natural_log_exp_and_others; strip
    them from every other set so the table-load assigner never thrashes."""
    from concourse import hw_specs

    tables = hw_specs.get_activation_tables(nc.m.arch)
    used = {AF.Copy, AF.Relu, AF.Square, AF.Abs_reciprocal_sqrt, AF.Identity}
    keep = "abs_reciprocal_sqrt_and_small"
    assert used <= tables[keep], (used - tables[keep])
    for name, s in tables.items():
        if name != keep:
            s.difference_update(used)


def build_program():
    nc = bacc.Bacc("TRN2", target_bir_lowering=False, debug=False, num_devices=M)
    _force_single_act_set(nc)

    # ---------------- DRAM I/O ----------------
    xT = nc.dram_tensor("xT", [ND, 128, C], F32R, kind="ExternalInput")
    ohT = nc.dram_tensor("ohT", [A, C], BF16, kind="ExternalInput")
    w1a_d = nc.dram_tensor("w1a", [ND, 128, H], F32R, kind="ExternalInput")
    w1b_d = nc.dram_tensor("w1b", [ND, 128, H], F32R, kind="ExternalInput")
    w1c_d = nc.dram_tensor("w1c", [ND, 128, H], F32R, kind="ExternalInput")
    n1x_d = nc.dram_tensor("n1x", [ND, 128, H], F32R, kind="ExternalInput")
    n1a_d = nc.dram_tensor("n1a", [A, H], BF16, kind="ExternalInput")
    ew2_d = nc.dram_tensor("ew2", [NH, 128, H], BF16, kind="ExternalInput")
    wsg_d = nc.dram_tensor("wsg", [NH, 128, H], BF16, kind="ExternalInput")
    nw2_d = nc.dram_tensor("nw2", [NH, 128, H], BF16, kind="ExternalInput")
    nw3_d = nc.dram_tensor("nw3", [NH, 128, D], BF16, kind="ExternalInput")
    yT = nc.dram_tensor("yT", [ND, 128, C], F32R, kind="ExternalOutput")

    with tile.TileContext(nc) as tc:
        with (
            tc.tile_pool(name="persist", bufs=1) as pp,
            tc.tile_pool(name="work_i", bufs=3) as pi,
            tc.tile_pool(name="stats", bufs=2) as ps,
            tc.tile_pool(name="work_n", bufs=1) as pn,
            tc.tile_pool(name="sred", bufs=2) as pr,
            tc.tile_pool(name="z2", bufs=2) as pz,
            tc.tile_pool(name="mm", bufs=3, space="PSUM") as pm,
            tc.tile_pool(name="stat_ps", bufs=2, space="PSUM") as pq,
        ):
            # ---------------- persistent SBUF ----------------
            flat_t = pp.tile([128, ND, C], F32R)
            node_t = pp.tile([128, ND, C], F32R)
            u_t = pp.tile([128, NH, EC], BF16)
            v_t = pp.tile([128, NH, EC], BF16)
            s_t = pp.tile([128, NH, C], BF16)
            m1_t = pp.tile([128, NH, C], BF16)
            m2_t = pp.tile([128, NH, C], BF16)
            w1a_t = pp.tile([128, ND, H], F32R)
            w1b_t = pp.tile([128, ND, H], F32R)
            w1c_t = pp.tile([128, ND, H], F32R)
            n1x_t = pp.tile([128, ND, H], F32R)
            n1a_t = pp.tile([128, H], BF16)  # rows [:A] used
            oh_t = pp.tile([128, C], BF16)  # rows [:A] used
            ew2_t = pp.tile([128, NH, H], BF16)
            wsg_t = pp.tile([128, NH, H], BF16)
            nw2_t = pp.tile([128, NH, H], BF16)
            nw3_t = pp.tile([128, NH, D], BF16)
            ones_t = pp.tile([128, 128], BF16)
            eps_t = pp.tile([128, 1], F32)

            nc.vector.memset(ones_t[:], 1.0 / H)
            nc.vector.memset(s_t[:, :, EC:C], 0.0)
            nc.vector.memset(eps_t[:], EPS)
            for k in range(ND):
                nc.sync.dma_start(flat_t[:, k, :], xT[k])
                nc.sync.dma_start(w1a_t[:, k, :], w1a_d[k])
                nc.sync.dma_start(w1c_t[:, k, :], w1c_d[k])
            for k in range(ND):
                nc.sync.dma_start(w1b_t[:, k, :], w1b_d[k])
                nc.sync.dma_start(n1x_t[:, k, :], n1x_d[k])
            for k in range(NH):
                nc.sync.dma_start(ew2_t[:, k, :], ew2_d[k])
                nc.sync.dma_start(wsg_t[:, k, :], wsg_d[k])
                nc.sync.dma_start(nw2_t[:, k, :], nw2_d[k])
                nc.sync.dma_start(nw3_t[:, k, :], nw3_d[k])
            nc.sync.dma_start(n1a_t[:A, :], n1a_d[:])
            nc.sync.dma_start(oh_t[:A, :], ohT[:])

            ECH = [(0, 512), (512, 384)]  # edge-col chunks
            NCH = [(0, 512), (512, 512)]  # node-col chunks (s zero-padded in tail)

            def emit_uv(include_node):
                """u = flat@w1a (+ node@w1b) -> bf16 u_t; v = flat@w1c -> v_t.
                Edge cols only."""
                for c0, cw in ECH:
                    for hp in range(2):
                        up = pm.tile([128, 2, 512], F32, tag="mm")
                        vp = pm.tile([128, 2, 512], F32, tag="mm")
                        for hh in range(2):
                            h = hp * 2 + hh
                            hs = slice(h * 128, (h + 1) * 128)
                            n_acc = ND * (2 if include_node else 1)
                            idx = 0
                            for k in range(ND):
                                nc.tensor.matmul(
                                    up[:, hh, :cw],
                                    lhsT=w1a_t[:, k, hs],
                                    rhs=flat_t[:, k, c0 : c0 + cw],
                                    start=(idx == 0),
                                    stop=(idx == n_acc - 1),
                                )
                                idx += 1
                            if include_node:
                                for k in range(ND):
                                    nc.tensor.matmul(
                                        up[:, hh, :cw],
                                        lhsT=w1b_t[:, k, hs],
                                        rhs=node_t[:, k, c0 : c0 + cw],
                                        start=(idx == 0),
                                        stop=(idx == n_acc - 1),
                                    )
                                    idx += 1
                            for k in range(ND):
                                nc.tensor.matmul(
                                    vp[:, hh, :cw],
                                    lhsT=w1c_t[:, k, hs],
                                    rhs=flat_t[:, k, c0 : c0 + cw],
                                    start=(k == 0),
                                    stop=(k == ND - 1),
                                )
                        nc.scalar.copy(
                            out=u_t[:, hp * 2 : hp * 2 + 2, c0 : c0 + cw],
                            in_=up[:, :, :cw],
                        )
                        nc.scalar.copy(
                            out=v_t[:, hp * 2 : hp * 2 + 2, c0 : c0 + cw],
                            in_=vp[:, :, :cw],
                        )

            def edge_stage_a(i):
                """h1 = relu(u_i + v_j); centered z2 psum -> z2b bf16."""
                W = S - 1  # 6 j-groups
                EW = W * 128  # 768
                h1 = pi.tile([128, NH, EW], BF16, tag="h1")
                for h in range(NH):
                    ui = u_t[:, h, i * 128 : (i + 1) * 128]
                    if i > 0:
                        nc.vector.tensor_tensor(
                            h1[:, h, 0 : i * 128].rearrange("p (j c) -> p j c", c=128),
                            v_t[:, h, 0 : i * 128].rearrange("p (j c) -> p j c", c=128),
                            _bc(ui, i, 128),
                            AL.add,
                        )
                    if i < W:
                        nj = W - i
                        nc.vector.tensor_tensor(
                            h1[:, h, i * 128 : EW].rearrange("p (j c) -> p j c", c=128),
                            v_t[:, h, (i + 1) * 128 : EC].rearrange(
                                "p (j c) -> p j c", c=128
                            ),
                            _bc(ui, nj, 128),
                            AL.add,
                        )
                nc.vector.tensor_scalar_max(h1[:, 0:2, :], h1[:, 0:2, :], 0.0)
                nc.vector.tensor_scalar_max(h1[:, 2:4, :], h1[:, 2:4, :], 0.0)

                z2b = pz.tile([128, NH, EW], BF16, tag="z2b")
                for ci, (c0, cw) in enumerate([(0, 384), (384, 384)]):
                    cs = slice(c0, c0 + cw)
                    for hp in range(2):
                        z2p = pm.tile([128, 2, 512], F32, tag="mm")
                        for hh in range(2):
                            h = hp * 2 + hh
                            hs = slice(h * 128, (h + 1) * 128)
                            for k in range(NH):
                                nc.tensor.matmul(
                                    z2p[:, hh, :cw],
                                    lhsT=ew2_t[:, k, hs],
                                    rhs=h1[:, k, cs],
                                    start=(k == 0),
                                    stop=(k == NH - 1),
                                )
                        nc.scalar.copy(
                            out=z2b[:, hp * 2 : hp * 2 + 2, cs], in_=z2p[:, :, :cw]
                        )
                return (z2b,)

            def edge_stage_b(i, z2b):
                """LN tail: t=z2c; var; rstd; h2=relu(t)*rstd; s-reduce."""
                EW = (S - 1) * 128
                t_e = pi.tile([128, NH, EW], BF16, tag="t_e")
                sq_e = pz.tile([128, NH, EW], BF16, tag="sq_e")
                nc.scalar.activation(sq_e[:, :, :], z2b[:, :, :], AF.Square)
                rstd = ps.tile([128, EW], BF16, tag="rstd")
                for ci, (c0, cw) in enumerate([(0, 384), (384, 384)]):
                    cs = slice(c0, c0 + cw)
                    var_ps = pq.tile([128, 512], F32, tag="stat")
                    for k in range(NH):
                        nc.tensor.matmul(
                            var_ps[:, :cw],
                            lhsT=ones_t[:],
                            rhs=sq_e[:, k, cs],
                            start=(k == 0),
                            stop=(k == NH - 1),
                        )
                    nc.scalar.activation(
                        rstd[:, cs],
                        var_ps[:, :cw],
                        AF.Abs_reciprocal_sqrt,
                        bias=eps_t[:, 0:1],
                        scale=1.0,
                    )
                # h2 = relu(z2c) * rstd, one fused DVE op
                nc.vector.scalar_tensor_tensor(
                    t_e[:, :, :],
                    z2b[:, :, :],
                    0.0,
                    _bc(rstd[:, :], NH, EW),
                    op0=AL.max,
                    op1=AL.mult,
                )
                eng = nc.vector if i >= S - 2 else nc.gpsimd
                for h in range(NH):
                    st1 = pr.tile([128, 384], BF16, tag="st1")
                    eng.tensor_add(st1[:, :], t_e[:, h, 0:384], t_e[:, h, 384:768])
                    st2 = pr.tile([128, 128], BF16, tag="st2")
                    eng.tensor_add(st2[:, :], st1[:, 0:128], st1[:, 128:256])
                    eng.tensor_add(
                        s_t[:, h, i * 128 : (i + 1) * 128], st2[:, :], st1[:, 256:384]
                    )

            def emit_m1(c, r):
                # m1 chunk = relu(flat@n1x + s@wsg (+ oh@n1a at r=0))
                if True:
                    c0, cw = NCH[c]
                    for hp in range(2):
                        m1p = pm.tile([128, 2, 512], F32, tag="mm")
                        for hh in range(2):
                            h = hp * 2 + hh
                            hs = slice(h * 128, (h + 1) * 128)
                            n_acc = ND + NH + (1 if r == 0 else 0)
                            idx = 0
                            for k in range(ND):
                                nc.tensor.matmul(
                                    m1p[:, hh, :cw],
                                    lhsT=n1x_t[:, k, hs],
                                    rhs=flat_t[:, k, c0 : c0 + cw],
                                    start=(idx == 0),
                                    stop=(idx == n_acc - 1),
                                )
                                idx += 1
                            for k in range(NH):
                                nc.tensor.matmul(
                                    m1p[:, hh, :cw],
                                    lhsT=wsg_t[:, k, hs],
                                    rhs=s_t[:, k, c0 : c0 + cw],
                                    start=(idx == 0),
                                    stop=(idx == n_acc - 1),
                                )
                                idx += 1
                            if r == 0:
                                nc.tensor.matmul(
                                    m1p[:, hh, :cw],
                                    lhsT=n1a_t[:A, hs],
                                    rhs=oh_t[:A, c0 : c0 + cw],
                                    start=(idx == 0),
                                    stop=(idx == n_acc - 1),
                                )
                                idx += 1
                        nc.scalar.activation(
                            m1_t[:, hp * 2 : hp * 2 + 2, c0 : c0 + cw],
                            m1p[:, :, :cw],
                            AF.Relu,
                        )

            def emit_node(r):
                # ---- pipelined chunk stages ----
                t_n = pn.tile([128, NH, C], BF16, tag="t_n")
                sq_n = pn.tile([128, NH, C], BF16, tag="sq_n")
                rstd_n = pn.tile([128, C], BF16, tag="rstd_n")

                def node_a(c):
                    c0, cw = NCH[c]
                    cs = slice(c0, c0 + cw)
                    for hp in range(2):
                        zp = pm.tile([128, 2, 512], F32, tag="mm")
                        for hh in range(2):
                            h = hp * 2 + hh
                            hs = slice(h * 128, (h + 1) * 128)
                            for k in range(NH):
                                nc.tensor.matmul(
                                    zp[:, hh, :cw],
                                    lhsT=nw2_t[:, k, hs],
                                    rhs=m1_t[:, k, cs],
                                    start=(k == 0),
                                    stop=(k == NH - 1),
                                )
                        nc.vector.tensor_copy(
                            out=t_n[:, hp * 2 : hp * 2 + 2, cs], in_=zp[:, :, :cw]
                        )

                def node_b(c):
                    c0, cw = NCH[c]
                    cs = slice(c0, c0 + cw)
                    nc.scalar.activation(sq_n[:, :, cs], t_n[:, :, cs], AF.Square)
                    var_ps = pq.tile([128, 512], F32, tag="stat")
                    for k in range(NH):
                        nc.tensor.matmul(
                            var_ps[:, :cw],
                            lhsT=ones_t[:],
                            rhs=sq_n[:, k, cs],
                            start=(k == 0),
                            stop=(k == NH - 1),
                        )
                    nc.scalar.activation(
                        rstd_n[:, cs],
                        var_ps[:, :cw],
                        AF.Abs_reciprocal_sqrt,
                        bias=eps_t[:, 0:1],
                        scale=1.0,
                    )
                    # m2 = relu(z)*rstd, fused
                    nc.vector.scalar_tensor_tensor(
                        m2_t[:, :, cs],
                        t_n[:, :, cs],
                        0.0,
                        _bc(rstd_n[:, cs], NH, cw),
                        op0=AL.max,
                        op1=AL.mult,
                    )

                def node_c(c):
                    c0, cw = NCH[c]
                    cs = slice(c0, c0 + cw)
                    np_ = pm.tile([128, 2, 512], F32, tag="mm")
                    for dd in range(ND):
                        ds_ = slice(dd * 128, (dd + 1) * 128)
                        for k in range(NH):
                            nc.tensor.matmul(
                                np_[:, dd, :cw],
                                lhsT=nw3_t[:, k, ds_],
                                rhs=m2_t[:, k, cs],
                                start=(k == 0),
                                stop=(k == NH - 1),
                            )
                    nc.scalar.copy(out=node_t[:, :, cs], in_=np_[:, :, :cw])
                    for k in range(ND):
                        nc.vector.tensor_tensor(
                            flat_t[:, k, cs], flat_t[:, k, cs], node_t[:, k, cs], AL.add
                        )
                    if r < ROUNDS - 1:
                        cw_uv = min(c0 + cw, EC) - c0
                        for hp in range(2):
                            up = pm.tile([128, 2, 512], F32, tag="mm")
                            vp = pm.tile([128, 2, 512], F32, tag="mm")
                            for hh in range(2):
                                h = hp * 2 + hh
                                hs = slice(h * 128, (h + 1) * 128)
                                for k in range(ND):
                                    nc.tensor.matmul(
                                        up[:, hh, :cw_uv],
                                        lhsT=w1a_t[:, k, hs],
                                        rhs=flat_t[:, k, c0 : c0 + cw_uv],
                                        start=(k == 0),
                                        stop=False,
                                    )
                                for k in range(ND):
                                    nc.tensor.matmul(
                                        up[:, hh, :cw_uv],
                                        lhsT=w1b_t[:, k, hs],
                                        rhs=node_t[:, k, c0 : c0 + cw_uv],
                                        start=False,
                                        stop=(k == ND - 1),
                                    )
                                for k in range(ND):
                                    nc.tensor.matmul(
                                        vp[:, hh, :cw_uv],
                                        lhsT=w1c_t[:, k, hs],
                                        rhs=flat_t[:, k, c0 : c0 + cw_uv],
                                        start=(k == 0),
                                        stop=(k == ND - 1),
                                    )
                            nc.scalar.copy(
                                out=v_t[:, hp * 2 : hp * 2 + 2, c0 : c0 + cw_uv],
                                in_=vp[:, :, :cw_uv],
                            )
                            nc.scalar.copy(
                                out=u_t[:, hp * 2 : hp * 2 + 2, c0 : c0 + cw_uv],
                                in_=up[:, :, :cw_uv],
                            )

                node_a(0)
                emit_m1(1, r)
                node_a(1)
                node_b(0)
                node_b(1)
                node_c(0)
                node_c(1)

            # ---------------- the 8 rounds ----------------
            emit_uv(include_node=False)
            for r in range(ROUNDS):
                pending = None
                for i in range(S):
                    ab = edge_stage_a(i)
                    if pending is not None:
                        edge_stage_b(i - 1, *pending)
                    pending = ab
                    if i == 6:
                        emit_m1(0, r)  # s slots 0-3 ready well before this
                edge_stage_b(S - 1, *pending)
                emit_node(r)  # node_c also emits next round's u/v inline

            for k in range(ND):
                nc.sync.dma_start(yT[k], flat_t[:, k, :])

    nc.compile()
    return nc


# ---------------------------------------------------------------------------
# Host side
# ---------------------------------------------------------------------------


def _host_prep(inputs):
    """Build per-core input maps.  Returns (in_maps, perm) where perm maps
    device column order back to global flat-row order."""
    states = np.asarray(inputs["states"], np.float32).reshape(B * N, D)
    action = np.asarray(inputs["action"]).astype(np.int64)

    e_w1 = np.asarray(inputs["e_w1"], np.float32)
    e_w2 = np.asarray(inputs["e_w2"], np.float32)
    e_w3 = np.asarray(inputs["e_w3"], np.float32)
    n_w1 = np.asarray(inputs["n_w1"], np.float32)
    n_w2 = np.asarray(inputs["n_w2"], np.float32)
    n_w3 = np.asarray(inputs["n_w3"], np.float32)

    w1a, w1b, w1c = e_w1[0:D], e_w1[D : 2 * D], e_w1[2 * D : 3 * D]
    n1x = n_w1[0:D]
    n1a = n_w1[D : D + A]
    n1g = n_w1[D + A :]
    wsg = e_w3 @ n1g  # [H, H]

    onehot = np.zeros((B, A), np.float32)
    onehot[np.arange(B), action] = 1.0

    # device column -> global flat row, per core
    perms = []
    for d in range(M):
        edge_rows = np.empty(EC, np.int64)
        for s in range(S):
            for b in range(NB):
                edge_rows[s * NB + b] = 896 * d + 7 * b + s
        tail_rows = np.arange(B * S + TC * d, B * S + TC * (d + 1), dtype=np.int64)
        perms.append(np.concatenate([edge_rows, tail_rows]))

    def kt(w, nk):  # [K, F] -> [nk, 128, F]
        return np.ascontiguousarray(w.reshape(nk, 128, -1))

    import ml_dtypes

    bf = np.float16
    f8 = ml_dtypes.float8_e4m3
    common = {
        "w1a": kt(w1a, ND),
        "w1b": kt(w1b, ND),
        "w1c": kt(w1c, ND),
        "n1x": kt(n1x, ND),
        "n1a": np.ascontiguousarray(n1a).astype(bf),
        # LN mean folded into the layer-2 weights (zero-bias fast path):
        # z2 - mean(z2) == h1 @ (w2 - rowsum(w2)/H), exactly.
        "ew2": kt(e_w2 - e_w2.sum(1, keepdims=True) / H, NH).astype(bf),
        "wsg": kt(wsg, NH).astype(bf),
        "nw2": kt(n_w2 - n_w2.sum(1, keepdims=True) / H, NH).astype(bf),
        "nw3": kt(n_w3, NH).astype(bf),
    }

    in_maps = []
    for d in range(M):
        rows = perms[d]
        xT_d = np.ascontiguousarray(states[rows].T.reshape(ND, 128, C))
        oh_d = np.ascontiguousarray(onehot[rows // N].T).astype(bf)  # [A, C]
        in_maps.append({"xT": xT_d, "ohT": oh_d, **common})
    return in_maps, perms


def _check_fast_path(inputs):
    z = lambda k: np.allclose(np.asarray(inputs[k]), 0.0)
    o = lambda k: np.allclose(np.asarray(inputs[k]), 1.0)
    return (
        z("e_b1") and z("e_b2") and z("e_bn") and z("e_b3")
        and z("n_b1") and z("n_b2") and z("n_bn") and z("n_b3")
        and o("e_g") and o("n_g")
    )


def _numpy_fallback(inputs):
    """Exact NumPy port of reference.py (used only if the fast-path
    assumptions about biases/LN-affine do not hold)."""
    f32 = np.float32
    states = np.asarray(inputs["states"], f32)
    action = np.asarray(inputs["action"]).astype(np.int64)
    g = {k: np.asarray(v, f32) for k, v in inputs.items() if k not in ("states", "action")}

    def ln(x, ga, be):
        m = x.mean(-1, keepdims=True)
        v = x.var(-1, keepdims=True)
        return (x - m) / np.sqrt(v + EPS) * ga + be

    def mlp(x, w1, b1, w2, b2, ga, bn, w3, b3):
        h = np.maximum(x @ w1 + b1, 0)
        h = np.maximum(ln(h @ w2 + b2, ga, bn), 0)
        return h @ w3 + b3

    eP = (g["e_w1"], g["e_b1"], g["e_w2"], g["e_b2"], g["e_g"], g["e_bn"], g["e_w3"], g["e_b3"])
    nP = (g["n_w1"], g["n_b1"], g["n_w2"], g["n_b2"], g["n_g"], g["n_bn"], g["n_w3"], g["n_b3"])
    flat = states.reshape(-1, D)
    pairs = np.array([(i, j) for i in range(S) for j in range(S) if i != j], np.int64)
    off = (np.arange(B, dtype=np.int64) * S)[:, None]
    row = (pairs[:, 0][None, :] + off).reshape(-1)
    col = (pairs[:, 1][None, :] + off).reshape(-1)
    E = row.shape[0]
    onehot = np.zeros((B, A), f32)
    onehot[np.arange(B), action] = 1.0
    av = np.repeat(onehot, N, axis=0)

    def seg_sum(e):
        agg = np.zeros((B * N, H), f32)
        np.add.at(agg, row, e)
        return agg

    e = mlp(np.concatenate([flat[row], np.zeros((E, D), f32), flat[col]], 1), *eP)
    node = mlp(np.concatenate([flat, av, seg_sum(e)], 1), *nP)
    flat = flat + node
    av0 = np.zeros_like(av)
    for _ in range(N - 1):
        e = mlp(np.concatenate([flat[row], node[row], flat[col]], 1), *eP)
        node = mlp(np.concatenate([flat, av0, seg_sum(e)], 1), *nP)
        flat = flat + node
    return flat.reshape(B, N, D).astype(np.float32)


def get_program():
    if "nc" not in _CACHE:
        _CACHE["nc"] = build_program()
    return _CACHE["nc"]


def kernel(**inputs):
    if not _check_fast_path(inputs):
        return _numpy_fallback(inputs)

    nc = get_program()
    in_maps, perms = _host_prep(inputs)
    res = bass_utils.run_bass_kernel_spmd(nc, in_maps, core_ids=list(range(M)))
    _CACHE["last_results"] = res

    out = np.empty((B * N, D), np.float32)
    for d in range(M):
        yT = res.results[d]["yT"].reshape(D, C)  # [D, C]
        out[perms[d]] = yT.T
    return out.reshape(B, N, D)


if __name__ == "__main__":
    rng = np.random.default_rng(0)
    print("building program...")
    nc = get_program()
    print("built.")



# revision 39
# speedup vs baseline: 1.1168x; 1.1168x over previous
"""Trainium2 Bass kernel for nn_ContrastiveSWM (GNN message passing).

Math (per reference.py):
  B=1024, N=8, D=256, H=512, A=4.  flat = states.reshape(B*N, D)
  Edge list over N-1=7 "virtual" objects with stride 7: flat rows
  [7g, 7g+7) form 1024 independent fully-connected 7-node blocks
  (rows 0..7167); rows 7168..8191 have no edges.
  8 rounds of: e = edgeMLP([flat[row], node[row], flat[col]]);
  agg = segsum(e, row); node = nodeMLP([flat, av, agg]); flat += node.

Key algebraic restructurings (exact, not approximations):
  * edge MLP layer 1 splits: u_i = flat_i@w1a + node_i@w1b (+b1),
    v_j = flat_j@w1c; per-edge preact = u_i + v_j.
  * edge MLP layer 3 + segment-sum commute: agg_i = (sum_j h2_ij)@e_w3
    (+ 6*e_b3), and e_w3 further folds into the node MLP:
    agg@n_w1g = s@(e_w3@n_w1g).  So only layer 2 is per-edge.
  * LN mean folds into the layer-2 weights (biases are zero in the fast
    path): z2 - mean = h1 @ (e_w2 - rowsum(e_w2)/H), exactly.  var =
    mean of (z2c)^2 via ones/H matmul, rstd in one ACT op
    (Abs_reciprocal_sqrt; single act-table set).

Sharding: data-parallel over the 1024 edge-blocks: core d owns blocks
[128d,128(d+1)) = flat rows [896d, 896(d+1)), plus tail rows
[7168+128d, 7168+128(d+1)).  Weights replicated.  No collectives.

On-device layout is feature-major (features on partitions, rows in the
free dim), slot-major columns (col = slot*128 + block) so that the
fully-connected gather becomes contiguous 128-column slices.

Dtypes: residual stream (flat/node) and layer-1 weights in float32r
(full-rate PE at free-dim>=256, ~tf32 accuracy); all other activations
and weights in float16 (fast DVE modes, 8x finer mantissa than bf16);
PSUM accumulation fp32.  Measured absmax-relative error vs the fp32
reference: 7.2e-4.  The edge loop is software-pipelined (stage B of
slot i emitted after stage A of slot i+1) because each engine executes
its instruction stream in order.
"""

import numpy as np

try:
    import concourse.bass as bass
except ImportError:  # environment fallback
    import sys

    sys.path.insert(0, "/opt/trn_rl_repo")
    import concourse.bass as bass

import concourse.mybir as mybir
import concourse.tile as tile
from concourse import bacc, bass_utils

F32 = mybir.dt.float32
F32R = mybir.dt.float32r
BF16 = mybir.dt.float16  # fp16: same speed class as bf16, 8x finer mantissa
F8 = mybir.dt.float8e4  # TRN e4m3 (max 240); enables DoubleRow 2x matmul
DR = mybir.MatmulPerfMode.DoubleRow
AL = mybir.AluOpType
AF = mybir.ActivationFunctionType

# fp8 scaling scheme (all exact, compensated):
#  * Square ops use scale=1/16 so sq values fit fp8e4 range.
#  * wsg, nw2 are scaled x16 on host (keeps fp8 weights out of denormals);
#    the LN rstd ACT op absorbs the compensation:
#      edge: rstd_stored = rstd/16  (so s=sum h2/16 pairs with 16*wsg)
#      node: t_n = 16*z2c, rstd_stored = rstd/16 (relu commutes with scale)
SQS = 0.0625  # square input prescale

B, N, D, H, A = 1024, 8, 256, 512, 4
M = 8  # cores
NB = 128  # blocks per core
S = N - 1  # 7 slots per block
EC = NB * S  # 896 edge cols per core
TC = (B * N - B * S) // M  # 128 tail cols per core
C = EC + TC  # 1024 cols per core
ROUNDS = N  # 8
EPS = 1e-5
ND = D // 128  # 2
NH = H // 128  # 4

_CACHE = {}


def _bc(ap2d, n, w):
    """[p, w] -> [p, n, w] broadcast along a new middle dim."""
    return ap2d.unsqueeze(1).to_broadcast((ap2d.shape[0], n, w))


def _force_single_act_set(nc):
    """All activation funcs we use live in natural_log_exp_and_others; strip
    them from every other set so the table-load assigner never thrashes."""
    from concourse import hw_specs

    tables = hw_specs.get_activation_tables(nc.m.arch)
    used = {AF.Copy, AF.Relu, AF.Square, AF.Abs_reciprocal_sqrt, AF.Identity}
    keep = "abs_reciprocal_sqrt_and_small"
    assert used <= tables[keep], (used - tables[keep])
    for name, s in tables.items():
        if name != keep:
            s.difference_update(used)


def build_program():
    nc = bacc.Bacc("TRN2", target_bir_lowering=False, debug=False, num_devices=M)
    _force_single_act_set(nc)

    # ---------------- DRAM I/O ----------------
    xT = nc.dram_tensor("xT", [ND, 128, C], F32R, kind="ExternalInput")
    ohT = nc.dram_tensor("ohT", [A, C], BF16, kind="ExternalInput")
    w1a_d = nc.dram_tensor("w1a", [ND, 128, H], F32R, kind="ExternalInput")
    w1b_d = nc.dram_tensor("w1b", [ND, 128, H], F32R, kind="ExternalInput")
    w1c_d = nc.dram_tensor("w1c", [ND, 128, H], F32R, kind="ExternalInput")
    n1x_d = nc.dram_tensor("n1x", [ND, 128, H], F32R, kind="ExternalInput")
    n1a_d = nc.dram_tensor("n1a", [A, H], BF16, kind="ExternalInput")
    ew2_d = nc.dram_tensor("ew2", [NH, 128, H], BF16, kind="ExternalInput")
    wsg_d = nc.dram_tensor("wsg", [NH, 128, H], BF16, kind="ExternalInput")
    nw2_d = nc.dram_tensor("nw2", [NH, 128, H], BF16, kind="ExternalInput")
    nw3_d = nc.dram_tensor("nw3", [NH, 128, D], BF16, kind="ExternalInput")
    yT = nc.dram_tensor("yT", [ND, 128, C], F32R, kind="ExternalOutput")

    with tile.TileContext(nc) as tc:
        with (
            tc.tile_pool(name="persist", bufs=1) as pp,
            tc.tile_pool(name="work_i", bufs=3) as pi,
            tc.tile_pool(name="stats", bufs=2) as ps,
            tc.tile_pool(name="work_n", bufs=1) as pn,
            tc.tile_pool(name="sred", bufs=2) as pr,
            tc.tile_pool(name="z2", bufs=2) as pz,
            tc.tile_pool(name="mm", bufs=3, space="PSUM") as pm,
            tc.tile_pool(name="stat_ps", bufs=2, space="PSUM") as pq,
        ):
            # ---------------- persistent SBUF ----------------
            flat_t = pp.tile([128, ND, C], F32R)
            node_t = pp.tile([128, ND, C], F32R)
            u_t = pp.tile([128, NH, EC], BF16)
            v_t = pp.tile([128, NH, EC], BF16)
            s_t = pp.tile([128, NH, C], BF16)
            m1_t = pp.tile([128, NH, C], BF16)
            m2_t = pp.tile([128, NH, C], BF16)
            w1a_t = pp.tile([128, ND, H], F32R)
            w1b_t = pp.tile([128, ND, H], F32R)
            w1c_t = pp.tile([128, ND, H], F32R)
            n1x_t = pp.tile([128, ND, H], F32R)
            n1a_t = pp.tile([128, H], BF16)  # rows [:A] used
            oh_t = pp.tile([128, C], BF16)  # rows [:A] used
            ew2_t = pp.tile([128, NH, H], BF16)
            wsg_t = pp.tile([128, NH, H], BF16)
            nw2_t = pp.tile([128, NH, H], BF16)
            nw3_t = pp.tile([128, NH, D], BF16)
            ones_t = pp.tile([128, 128], BF16)
            eps_t = pp.tile([128, 1], F32)

            nc.vector.memset(ones_t[:], 1.0 / H)
            nc.vector.memset(s_t[:, :, EC:C], 0.0)
            nc.vector.memset(eps_t[:], EPS)
            for k in range(ND):
                nc.sync.dma_start(flat_t[:, k, :], xT[k])
                nc.sync.dma_start(w1a_t[:, k, :], w1a_d[k])
                nc.sync.dma_start(w1c_t[:, k, :], w1c_d[k])
            for k in range(ND):
                nc.sync.dma_start(w1b_t[:, k, :], w1b_d[k])
                nc.sync.dma_start(n1x_t[:, k, :], n1x_d[k])
            for k in range(NH):
                nc.sync.dma_start(ew2_t[:, k, :], ew2_d[k])
                nc.sync.dma_start(wsg_t[:, k, :], wsg_d[k])
                nc.sync.dma_start(nw2_t[:, k, :], nw2_d[k])
                nc.sync.dma_start(nw3_t[:, k, :], nw3_d[k])
            nc.sync.dma_start(n1a_t[:A, :], n1a_d[:])
            nc.sync.dma_start(oh_t[:A, :], ohT[:])

            ECH = [(0, 512), (512, 384)]  # edge-col chunks
            NCH = [(0, 512), (512, 512)]  # node-col chunks (s zero-padded in tail)

            def emit_uv(include_node):
                """u = flat@w1a (+ node@w1b) -> bf16 u_t; v = flat@w1c -> v_t.
                Edge cols only."""
                for c0, cw in ECH:
                    for hp in range(2):
                        up = pm.tile([128, 2, 512], F32, tag="mm")
                        vp = pm.tile([128, 2, 512], F32, tag="mm")
                        for hh in range(2):
                            h = hp * 2 + hh
                            hs = slice(h * 128, (h + 1) * 128)
                            n_acc = ND * (2 if include_node else 1)
                            idx = 0
                            for k in range(ND):
                                nc.tensor.matmul(
                                    up[:, hh, :cw],
                                    lhsT=w1a_t[:, k, hs],
                                    rhs=flat_t[:, k, c0 : c0 + cw],
                                    start=(idx == 0),
                                    stop=(idx == n_acc - 1),
                                )
                                idx += 1
                            if include_node:
                                for k in range(ND):
                                    nc.tensor.matmul(
                                        up[:, hh, :cw],
                                        lhsT=w1b_t[:, k, hs],
                                        rhs=node_t[:, k, c0 : c0 + cw],
                                        start=(idx == 0),
                                        stop=(idx == n_acc - 1),
                                    )
                                    idx += 1
                            for k in range(ND):
                                nc.tensor.matmul(
                                    vp[:, hh, :cw],
                                    lhsT=w1c_t[:, k, hs],
                                    rhs=flat_t[:, k, c0 : c0 + cw],
                                    start=(k == 0),
                                    stop=(k == ND - 1),
                                )
                        nc.scalar.copy(
                            out=u_t[:, hp * 2 : hp * 2 + 2, c0 : c0 + cw],
                            in_=up[:, :, :cw],
                        )
                        nc.scalar.copy(
                            out=v_t[:, hp * 2 : hp * 2 + 2, c0 : c0 + cw],
                            in_=vp[:, :, :cw],
                        )

            def edge_stage_dve(i):
                """h1 = relu(u_i + v_j) on DVE only (emitted one slot ahead
                so PE's z2(i) never waits on the h1 build)."""
                W = S - 1  # 6 j-groups
                EW = W * 128  # 768
                h1 = pi.tile([128, NH, EW], BF16, tag="h1")
                for h in range(NH):
                    ui = u_t[:, h, i * 128 : (i + 1) * 128]
                    if i > 0:
                        nc.vector.tensor_tensor(
                            h1[:, h, 0 : i * 128].rearrange("p (j c) -> p j c", c=128),
                            v_t[:, h, 0 : i * 128].rearrange("p (j c) -> p j c", c=128),
                            _bc(ui, i, 128),
                            AL.add,
                        )
                    if i < W:
                        nj = W - i
                        nc.vector.tensor_tensor(
                            h1[:, h, i * 128 : EW].rearrange("p (j c) -> p j c", c=128),
                            v_t[:, h, (i + 1) * 128 : EC].rearrange(
                                "p (j c) -> p j c", c=128
                            ),
                            _bc(ui, nj, 128),
                            AL.add,
                        )
                nc.vector.tensor_scalar_max(h1[:, 0:2, :], h1[:, 0:2, :], 0.0)
                nc.vector.tensor_scalar_max(h1[:, 2:4, :], h1[:, 2:4, :], 0.0)
                return h1

            def edge_stage_pe(i, h1):
                """centered z2 psum -> z2b bf16."""
                EW = (S - 1) * 128
                z2b = pz.tile([128, NH, EW], BF16, tag="z2b")
                for ci, (c0, cw) in enumerate([(0, 384), (384, 384)]):
                    cs = slice(c0, c0 + cw)
                    for hp in range(2):
                        z2p = pm.tile([128, 2, 512], F32, tag="mm")
                        for hh in range(2):
                            h = hp * 2 + hh
                            hs = slice(h * 128, (h + 1) * 128)
                            for k in range(NH):
                                nc.tensor.matmul(
                                    z2p[:, hh, :cw],
                                    lhsT=ew2_t[:, k, hs],
                                    rhs=h1[:, k, cs],
                                    start=(k == 0),
                                    stop=(k == NH - 1),
                                )
                        nc.scalar.copy(
                            out=z2b[:, hp * 2 : hp * 2 + 2, cs], in_=z2p[:, :, :cw]
                        )
                return z2b

            def edge_stage_b(i, z2b):
                """LN tail: t=z2c; var; rstd; h2=relu(t)*rstd; s-reduce."""
                EW = (S - 1) * 128
                t_e = pi.tile([128, NH, EW], BF16, tag="t_e")
                sq_e = pz.tile([128, NH, EW], BF16, tag="sq_e")
                nc.vector.tensor_tensor(
                    sq_e[:, 0:2, :], z2b[:, 0:2, :], z2b[:, 0:2, :], AL.mult
                )
                nc.vector.tensor_tensor(
                    sq_e[:, 2:4, :], z2b[:, 2:4, :], z2b[:, 2:4, :], AL.mult
                )
                rstd = ps.tile([128, EW], BF16, tag="rstd")
                for ci, (c0, cw) in enumerate([(0, 384), (384, 384)]):
                    cs = slice(c0, c0 + cw)
                    var_ps = pq.tile([128, 512], F32, tag="stat")
                    for k in range(NH):
                        nc.tensor.matmul(
                            var_ps[:, :cw],
                            lhsT=ones_t[:],
                            rhs=sq_e[:, k, cs],
                            start=(k == 0),
                            stop=(k == NH - 1),
                        )
                    nc.scalar.activation(
                        rstd[:, cs],
                        var_ps[:, :cw],
                        AF.Abs_reciprocal_sqrt,
                        bias=eps_t[:, 0:1],
                        scale=1.0,
                    )
                nc.vector.tensor_scalar_max(t_e[:, :, :], z2b[:, :, :], 0.0)
                nc.vector.tensor_tensor(
                    t_e[:, :, :], t_e[:, :, :], _bc(rstd[:, :], NH, EW), AL.mult
                )
                eng = nc.vector if i >= S - 2 else nc.gpsimd
                for h in range(NH):
                    st1 = pr.tile([128, 384], BF16, tag="st1")
                    eng.tensor_add(st1[:, :], t_e[:, h, 0:384], t_e[:, h, 384:768])
                    st2 = pr.tile([128, 128], BF16, tag="st2")
                    eng.tensor_add(st2[:, :], st1[:, 0:128], st1[:, 128:256])
                    eng.tensor_add(
                        s_t[:, h, i * 128 : (i + 1) * 128], st2[:, :], st1[:, 256:384]
                    )

            def emit_m1(c, r):
                # m1 chunk = relu(flat@n1x + s@wsg (+ oh@n1a at r=0))
                if True:
                    c0, cw = NCH[c]
                    for hp in range(2):
                        m1p = pm.tile([128, 2, 512], F32, tag="mm")
                        for hh in range(2):
                            h = hp * 2 + hh
                            hs = slice(h * 128, (h + 1) * 128)
                            n_acc = ND + NH + (1 if r == 0 else 0)
                            idx = 0
                            for k in range(ND):
                                nc.tensor.matmul(
                                    m1p[:, hh, :cw],
                                    lhsT=n1x_t[:, k, hs],
                                    rhs=flat_t[:, k, c0 : c0 + cw],
                                    start=(idx == 0),
                                    stop=(idx == n_acc - 1),
                                )
                                idx += 1
                            for k in range(NH):
                                nc.tensor.matmul(
                                    m1p[:, hh, :cw],
                                    lhsT=wsg_t[:, k, hs],
                                    rhs=s_t[:, k, c0 : c0 + cw],
                                    start=(idx == 0),
                                    stop=(idx == n_acc - 1),
                                )
                                idx += 1
                            if r == 0:
                                nc.tensor.matmul(
                                    m1p[:, hh, :cw],
                                    lhsT=n1a_t[:A, hs],
                                    rhs=oh_t[:A, c0 : c0 + cw],
                                    start=(idx == 0),
                                    stop=(idx == n_acc - 1),
                                )
                                idx += 1
                        nc.scalar.activation(
                            m1_t[:, hp * 2 : hp * 2 + 2, c0 : c0 + cw],
                            m1p[:, :, :cw],
                            AF.Relu,
                        )

            def emit_node(r):
                # ---- pipelined chunk stages ----
                t_n = pn.tile([128, NH, C], BF16, tag="t_n")
                sq_n = pn.tile([128, NH, C], BF16, tag="sq_n")
                rstd_n = pn.tile([128, C], BF16, tag="rstd_n")

                def node_a(c):
                    c0, cw = NCH[c]
                    cs = slice(c0, c0 + cw)
                    for hp in range(2):
                        zp = pm.tile([128, 2, 512], F32, tag="mm")
                        for hh in range(2):
                            h = hp * 2 + hh
                            hs = slice(h * 128, (h + 1) * 128)
                            for k in range(NH):
                                nc.tensor.matmul(
                                    zp[:, hh, :cw],
                                    lhsT=nw2_t[:, k, hs],
                                    rhs=m1_t[:, k, cs],
                                    start=(k == 0),
                                    stop=(k == NH - 1),
                                )
                        nc.vector.tensor_copy(
                            out=t_n[:, hp * 2 : hp * 2 + 2, cs], in_=zp[:, :, :cw]
                        )

                def node_b(c):
                    c0, cw = NCH[c]
                    cs = slice(c0, c0 + cw)
                    nc.scalar.activation(sq_n[:, :, cs], t_n[:, :, cs], AF.Square)
                    var_ps = pq.tile([128, 512], F32, tag="stat")
                    for k in range(NH):
                        nc.tensor.matmul(
                            var_ps[:, :cw],
                            lhsT=ones_t[:],
                            rhs=sq_n[:, k, cs],
                            start=(k == 0),
                            stop=(k == NH - 1),
                        )
                    nc.scalar.activation(
                        rstd_n[:, cs],
                        var_ps[:, :cw],
                        AF.Abs_reciprocal_sqrt,
                        bias=eps_t[:, 0:1],
                        scale=1.0,
                    )
                    nc.vector.tensor_scalar_max(t_n[:, :, cs], t_n[:, :, cs], 0.0)
                    nc.vector.tensor_tensor(
                        m2_t[:, :, cs], t_n[:, :, cs], _bc(rstd_n[:, cs], NH, cw), AL.mult
                    )

                def node_c(c):
                    c0, cw = NCH[c]
                    cs = slice(c0, c0 + cw)
                    np_ = pm.tile([128, 2, 512], F32, tag="mm")
                    for dd in range(ND):
                        ds_ = slice(dd * 128, (dd + 1) * 128)
                        for k in range(NH):
                            nc.tensor.matmul(
                                np_[:, dd, :cw],
                                lhsT=nw3_t[:, k, ds_],
                                rhs=m2_t[:, k, cs],
                                start=(k == 0),
                                stop=(k == NH - 1),
                            )
                    nc.scalar.copy(out=node_t[:, :, cs], in_=np_[:, :, :cw])
                    for k in range(ND):
                        nc.vector.tensor_tensor(
                            flat_t[:, k, cs], flat_t[:, k, cs], node_t[:, k, cs], AL.add
                        )
                    if r < ROUNDS - 1:
                        cw_uv = min(c0 + cw, EC) - c0
                        for hp in range(2):
                            up = pm.tile([128, 2, 512], F32, tag="mm")
                            vp = pm.tile([128, 2, 512], F32, tag="mm")
                            for hh in range(2):
                                h = hp * 2 + hh
                                hs = slice(h * 128, (h + 1) * 128)
                                for k in range(ND):
                                    nc.tensor.matmul(
                                        up[:, hh, :cw_uv],
                                        lhsT=w1a_t[:, k, hs],
                                        rhs=flat_t[:, k, c0 : c0 + cw_uv],
                                        start=(k == 0),
                                        stop=False,
                                    )
                                for k in range(ND):
                                    nc.tensor.matmul(
                                        up[:, hh, :cw_uv],
                                        lhsT=w1b_t[:, k, hs],
                                        rhs=node_t[:, k, c0 : c0 + cw_uv],
                                        start=False,
                                        stop=(k == ND - 1),
                                    )
                                for k in range(ND):
                                    nc.tensor.matmul(
                                        vp[:, hh, :cw_uv],
                                        lhsT=w1c_t[:, k, hs],
                                        rhs=flat_t[:, k, c0 : c0 + cw_uv],
                                        start=(k == 0),
                                        stop=(k == ND - 1),
                                    )
                            nc.scalar.copy(
                                out=v_t[:, hp * 2 : hp * 2 + 2, c0 : c0 + cw_uv],
                                in_=vp[:, :, :cw_uv],
                            )
                            nc.scalar.copy(
                                out=u_t[:, hp * 2 : hp * 2 + 2, c0 : c0 + cw_uv],
                                in_=up[:, :, :cw_uv],
                            )

                node_a(0)
                emit_m1(1, r)
                node_a(1)
                node_b(0)
                node_b(1)
                node_c(0)
                if r == ROUNDS - 1:
                    for k in range(ND):
                        nc.sync.dma_start(yT[k, :, 0:512], flat_t[:, k, 0:512])
                node_c(1)

            # ---------------- the 8 rounds ----------------
            emit_uv(include_node=False)
            for r in range(ROUNDS):
                h1_next = edge_stage_dve(0)
                pending = None
                for i in range(S):
                    h1_cur = h1_next
                    if i + 1 < S:
                        h1_next = edge_stage_dve(i + 1)  # one slot ahead
                    z2b = edge_stage_pe(i, h1_cur)
                    if pending is not None:
                        edge_stage_b(i - 1, pending)
                    pending = z2b
                    if i == 6:
                        emit_m1(0, r)
                edge_stage_b(S - 1, pending)
                emit_node(r)  # node_c also emits next round's u/v inline

            for k in range(ND):
                nc.sync.dma_start(yT[k, :, 512:C], flat_t[:, k, 512:C])

    nc.compile()
    return nc


# ---------------------------------------------------------------------------
# Host side
# ---------------------------------------------------------------------------


def _host_prep(inputs):
    """Build per-core input maps.  Returns (in_maps, perm) where perm maps
    device column order back to global flat-row order."""
    states = np.asarray(inputs["states"], np.float32).reshape(B * N, D)
    action = np.asarray(inputs["action"]).astype(np.int64)

    e_w1 = np.asarray(inputs["e_w1"], np.float32)
    e_w2 = np.asarray(inputs["e_w2"], np.float32)
    e_w3 = np.asarray(inputs["e_w3"], np.float32)
    n_w1 = np.asarray(inputs["n_w1"], np.float32)
    n_w2 = np.asarray(inputs["n_w2"], np.float32)
    n_w3 = np.asarray(inputs["n_w3"], np.float32)

    w1a, w1b, w1c = e_w1[0:D], e_w1[D : 2 * D], e_w1[2 * D : 3 * D]
    n1x = n_w1[0:D]
    n1a = n_w1[D : D + A]
    n1g = n_w1[D + A :]
    wsg = e_w3 @ n1g  # [H, H]

    onehot = np.zeros((B, A), np.float32)
    onehot[np.arange(B), action] = 1.0

    # device column -> global flat row, per core
    perms = []
    for d in range(M):
        edge_rows = np.empty(EC, np.int64)
        for s in range(S):
            for b in range(NB):
                edge_rows[s * NB + b] = 896 * d + 7 * b + s
        tail_rows = np.arange(B * S + TC * d, B * S + TC * (d + 1), dtype=np.int64)
        perms.append(np.concatenate([edge_rows, tail_rows]))

    def kt(w, nk):  # [K, F] -> [nk, 128, F]
        return np.ascontiguousarray(w.reshape(nk, 128, -1))

    import ml_dtypes

    bf = np.float16
    f8 = ml_dtypes.float8_e4m3
    common = {
        "w1a": kt(w1a, ND),
        "w1b": kt(w1b, ND),
        "w1c": kt(w1c, ND),
        "n1x": kt(n1x, ND),
        "n1a": np.ascontiguousarray(n1a).astype(bf),
        # LN mean folded into the layer-2 weights (zero-bias fast path):
        # z2 - mean(z2) == h1 @ (w2 - rowsum(w2)/H), exactly.
        "ew2": kt(e_w2 - e_w2.sum(1, keepdims=True) / H, NH).astype(bf),
        "wsg": kt(wsg, NH).astype(bf),
        "nw2": kt(n_w2 - n_w2.sum(1, keepdims=True) / H, NH).astype(bf),
        "nw3": kt(n_w3, NH).astype(bf),
    }

    in_maps = []
    for d in range(M):
        rows = perms[d]
        xT_d = np.ascontiguousarray(states[rows].T.reshape(ND, 128, C))
        oh_d = np.ascontiguousarray(onehot[rows // N].T).astype(bf)  # [A, C]
        in_maps.append({"xT": xT_d, "ohT": oh_d, **common})
    return in_maps, perms


def _check_fast_path(inputs):
    z = lambda k: np.allclose(np.asarray(inputs[k]), 0.0)
    o = lambda k: np.allclose(np.asarray(inputs[k]), 1.0)
    return (
        z("e_b1") and z("e_b2") and z("e_bn") and z("e_b3")
        and z("n_b1") and z("n_b2") and z("n_bn") and z("n_b3")
        and o("e_g") and o("n_g")
    )


def _numpy_fallback(inputs):
    """Exact NumPy port of reference.py (used only if the fast-path
    assumptions about biases/LN-affine do not hold)."""
    f32 = np.float32
    states = np.asarray(inputs["states"], f32)
    action = np.asarray(inputs["action"]).astype(np.int64)
    g = {k: np.asarray(v, f32) for k, v in inputs.items() if k not in ("states", "action")}

    def ln(x, ga, be):
        m = x.mean(-1, keepdims=True)
        v = x.var(-1, keepdims=True)
        return (x - m) / np.sqrt(v + EPS) * ga + be

    def mlp(x, w1, b1, w2, b2, ga, bn, w3, b3):
        h = np.maximum(x @ w1 + b1, 0)
        h = np.maximum(ln(h @ w2 + b2, ga, bn), 0)
        return h @ w3 + b3

    eP = (g["e_w1"], g["e_b1"], g["e_w2"], g["e_b2"], g["e_g"], g["e_bn"], g["e_w3"], g["e_b3"])
    nP = (g["n_w1"], g["n_b1"], g["n_w2"], g["n_b2"], g["n_g"], g["n_bn"], g["n_w3"], g["n_b3"])
    flat = states.reshape(-1, D)
    pairs = np.array([(i, j) for i in range(S) for j in range(S) if i != j], np.int64)
    off = (np.arange(B, dtype=np.int64) * S)[:, None]
    row = (pairs[:, 0][None, :] + off).reshape(-1)
    col = (pairs[:, 1][None, :] + off).reshape(-1)
    E = row.shape[0]
    onehot = np.zeros((B, A), f32)
    onehot[np.arange(B), action] = 1.0
    av = np.repeat(onehot, N, axis=0)

    def seg_sum(e):
        agg = np.zeros((B * N, H), f32)
        np.add.at(agg, row, e)
        return agg

    e = mlp(np.concatenate([flat[row], np.zeros((E, D), f32), flat[col]], 1), *eP)
    node = mlp(np.concatenate([flat, av, seg_sum(e)], 1), *nP)
    flat = flat + node
    av0 = np.zeros_like(av)
    for _ in range(N - 1):
        e = mlp(np.concatenate([flat[row], node[row], flat[col]], 1), *eP)
        node = mlp(np.concatenate([flat, av0, seg_sum(e)], 1), *nP)
        flat = flat + node
    return flat.reshape(B, N, D).astype(np.float32)


def get_program():
    if "nc" not in _CACHE:
        _CACHE["nc"] = build_program()
    return _CACHE["nc"]


def kernel(**inputs):
    if not _check_fast_path(inputs):
        return _numpy_fallback(inputs)

    nc = get_program()
    in_maps, perms = _host_prep(inputs)
    res = bass_utils.run_bass_kernel_spmd(nc, in_maps, core_ids=list(range(M)))
    _CACHE["last_results"] = res

    out = np.empty((B * N, D), np.float32)
    for d in range(M):
        yT = res.results[d]["yT"].reshape(D, C)  # [D, C]
        out[perms[d]] = yT.T
    return out.reshape(B, N, D)


if __name__ == "__main__":
    rng = np.random.default_rng(0)
    print("building program...")
    nc = get_program()
    print("built.")

